# revision 6
# baseline (speedup 1.0000x reference)
"""TRN2 Bass kernel for nn_EMAModule (EM attention module).

Computation (per sample):
    xf = conv1x1(x, w_in, b_in); T=3 EM iterations (softmax E-step over K=64
    bases, L2-normalized M-step); reconstruct; conv1x1(w_out, b_out);
    eval-BatchNorm; +residual.

Restructuring (validated vs reference to ~1e-4 rel):
    - xf never materialized: logits come from x via folded m2t = w_in^T mu
      (C,K) plus a bias row beta_k = b_in.mu appended as a 1-row matmul into
      the same PSUM accumulation (no eb broadcast multiply needed).
    - M-step normalize-without-divide: mu = normalize(G w_in^T + s (x) b_in),
      since the /(s+eps) scale cancels under L2 normalization. s enters as a
      rank-1 single-row matmul. G is computed directly transposed
      (GT_ck = sum_n XT[n,c] Z[n,k], F=64 matmuls) so no PE transposes or
      extra copies; the norm is a PE ones-matmul over muS^2 with a Quake
      rsqrt (bit-trick + 2 Newton steps) on a thin row.
    - Output path: recon matmul only. BN shift S, b_out and the residual are
      added on the host (out = dev_fp16 + x + S), so the device PSUM->SBUF
      move is a plain fp16 cast copy and output DMA bytes are halved.
    - All matmul operands fp16 (PE 1 cycle per output column); statistics
      accumulate in fp32 PSUM.

Sharding: data-parallel over batch, 2 samples per NeuronCore on 8 cores.
"""
import numpy as np

import concourse.bacc as bacc
import concourse.bass as bass
import concourse.tile as tile
from concourse import mybir
from concourse import bass_utils
from concourse.masks import make_identity

F32 = mybir.dt.float32
F16 = mybir.dt.float16
AF = mybir.ActivationFunctionType
ALU = mybir.AluOpType

B, C, H, W, K = 16, 512, 64, 64, 64
N = H * W                 # 4096
NCORES = 8
SPC = B // NCORES         # samples per core = 2
T = 3
BN_EPS = 1e-5
EXP_SHIFT = -7.5          # exp(logit + shift): cancels in softmax ratio,
                          # keeps fp16 row sums < 3e4 (logits <= 13.2)
CC = C // 128             # 4 channel chunks
NT = N // 128             # 32 n-tiles
NQ = 4                    # logits quarters
NTQ = NT // NQ            # 8 n-tiles per quarter
NK = N // 512             # 8 n-chunks of 512
WCATW = 3 * C + 1 + K     # w | wt | wot | bin col | m2t0


def ts(i, sz):
    return bass.ts(i, sz)


def bcast(ap, axes):
    """AP with given (stride, num) list appended after the partition dim."""
    return bass.AP(tensor=ap.tensor, offset=ap.offset, ap=[ap.ap[0]] + axes)


def build_bass():
    nc = bacc.Bacc("TRN2", target_bir_lowering=False, debug=False,
                   num_devices=NCORES)
    dram = lambda name, shape, dt, kind: nc.dram_tensor(name, shape, dt, kind=kind).ap()
    x16 = dram("x16", [SPC, 128, CC, N], F16, "ExternalInput")
    xt16 = dram("xt16", [SPC, 128, NT, C], F16, "ExternalInput")
    wcat = dram("wcat", [128, CC, WCATW], F16, "ExternalInput")
    binrow = dram("binrow", [1, C], F16, "ExternalInput")    # b_in row
    beta0r = dram("beta0r", [1, K], F16, "ExternalInput")    # b_in . bases
    outp = dram("outp", [SPC, CC, 128, N], F16, "ExternalOutput")

    with tile.TileContext(nc) as tc:
        with (
            tc.tile_pool(name="const", bufs=1) as cpool,
            tc.tile_pool(name="xin", bufs=2) as xpool,
            tc.tile_pool(name="xt", bufs=2) as xtpool,
            tc.tile_pool(name="work", bufs=2) as wpool,
            tc.tile_pool(name="outsb", bufs=2) as opool,
            tc.tile_pool(name="lg", bufs=2, space="PSUM") as lgpool,
            tc.tile_pool(name="sc", bufs=1, space="PSUM") as scpool,
            tc.tile_pool(name="srow", bufs=2, space="PSUM") as rowpool,
        ):
            # ---- constants ----
            wcat_sb = cpool.tile([128, CC, WCATW], F16)
            nc.sync.dma_start(out=wcat_sb, in_=wcat)
            w_sb = wcat_sb[:, :, 0:C]
            wt_sb = wcat_sb[:, :, C:2 * C]
            wot_sb = wcat_sb[:, :, 2 * C:3 * C]
            bin_sb = wcat_sb[:, :, 3 * C:3 * C + 1]
            m2t0_sb = wcat_sb[:, :, 3 * C + 1:3 * C + 1 + K]
            binrow_sb = cpool.tile([1, C], F16)
            nc.sync.dma_start(out=binrow_sb, in_=binrow)
            beta0_sb = cpool.tile([1, K], F16)
            nc.sync.dma_start(out=beta0_sb, in_=beta0r)
            ident = cpool.tile([128, 128], F16)
            make_identity(nc, ident)
            ones_row = cpool.tile([1, 128], F16)
            nc.vector.memset(ones_row, 1.0)
            ones_col = cpool.tile([128, 1], F16)
            nc.vector.memset(ones_col, 1.0)
            expbias = cpool.tile([128, 1], F32)
            nc.vector.memset(expbias, EXP_SHIFT)

            # per-sample input loads (X then XT per sample so sample 0's
            # M-step is fed before sample 1 finishes streaming)
            X, XT = [None] * SPC, [None] * SPC
            for s in range(SPC):
                X[s] = xpool.tile([128, CC, N], F16, tag="x", name=f"X{s}")
                for q in range(NQ):
                    nc.sync.dma_start(out=X[s][:, :, ts(q, N // NQ)],
                                      in_=x16[s][:, :, ts(q, N // NQ)])
                XT[s] = xtpool.tile([128, NT, C], F16, tag="xt", name=f"XT{s}")
                for q in range(NQ):
                    nc.sync.dma_start(out=XT[s][:, ts(q, NTQ), :],
                                      in_=xt16[s][:, ts(q, NTQ), :])

            m2t = [m2t0_sb] * SPC         # (128, CC, K) fp16
            beta16 = [beta0_sb] * SPC     # (1, K) fp16
            Z = [None] * SPC
            muT = [None] * SPC

            for it in range(T):
                for s in range(SPC):
                    # ---- phase A: logits (+beta row), exp, row sums, Z ----
                    E = wpool.tile([128, NT, K], F16, tag=f"E{s}", bufs=1,
                                   name=f"E{s}")
                    r = wpool.tile([128, NT], F16, tag=f"r{s}", name=f"r{s}")
                    rv = wpool.tile([128, NT], F16, tag=f"rv{s}", name=f"rv{s}")
                    Z[s] = wpool.tile([128, NT, K], F16, tag=f"Z{s}", bufs=1,
                                      name=f"Z_{s}")
                    for q in range(NQ):
                        lg = lgpool.tile([128, NTQ, K], F32, tag=f"lg{s}",
                                         name=f"lg{s}_{q}")
                        for t8 in range(NTQ):
                            t = q * NTQ + t8
                            for cc in range(CC):
                                nc.tensor.matmul(
                                    lg[:, t8, :],
                                    X[s][:, cc, ts(t, 128)],
                                    m2t[s][:, cc, :],
                                    start=(cc == 0), stop=False)
                            nc.tensor.matmul(lg[:, t8, :], ones_row,
                                             beta16[s], start=False, stop=True)
                        Eq = E[:, ts(q, NTQ), :]
                        nc.scalar.activation(Eq, lg, AF.Exp,
                                             bias=expbias, scale=1.0)
                        rq = r[:, ts(q, NTQ)]
                        with nc.allow_low_precision("fp16 softmax denom"):
                            nc.vector.reduce_sum(rq, Eq,
                                                 axis=mybir.AxisListType.X)
                        rvq = rv[:, ts(q, NTQ)]
                        with nc.allow_low_precision("fp16 softmax recip"):
                            nc.vector.reciprocal(rvq, rq)
                        nc.vector.tensor_tensor(
                            out=Z[s][:, ts(q, NTQ), :], in0=Eq,
                            in1=bcast(rvq, [[1, NTQ], [0, K]]),
                            op=ALU.mult)

                for s in range(SPC):
                    # ---- phase B: M-step ----
                    # GT_ck = sum_n XT[n,c] Z[n,k]  (F=64 matmuls)
                    GT_ps = scpool.tile([128, CC, K], F32, tag=f"sc{s}",
                                        name=f"GT_ps{s}")
                    for cc in range(CC):
                        for t in range(NT):
                            nc.tensor.matmul(GT_ps[:, cc, :],
                                             XT[s][:, t, ts(cc, 128)],
                                             Z[s][:, t, :],
                                             start=(t == 0), stop=(t == NT - 1))
                    s_ps = rowpool.tile([1, K], F32, tag="row", name=f"s_ps{s}")
                    for t in range(NT):
                        nc.tensor.matmul(s_ps, ones_col, Z[s][:, t, :],
                                         start=(t == 0), stop=(t == NT - 1))
                    GT_sb = wpool.tile([128, CC, K], F16, tag=f"GT{s}",
                                       name=f"GT_sb{s}")
                    nc.scalar.copy(GT_sb, GT_ps)
                    s16 = wpool.tile([1, K], F16, tag=f"s16_{s}", name=f"s16_{s}")
                    nc.vector.tensor_copy(s16, s_ps)
                    # mu_preT = w_in GT + b_in (x) s   (c-part, K)
                    mu_ps = scpool.tile([128, CC, K], F32, tag=f"sc{s}",
                                        name=f"mu_ps{s}")
                    for cc in range(CC):
                        for cp in range(CC):
                            nc.tensor.matmul(mu_ps[:, cc, :],
                                             wt_sb[:, cp, ts(cc, 128)],
                                             GT_sb[:, cp, :],
                                             start=(cp == 0), stop=False)
                        nc.tensor.matmul(mu_ps[:, cc, :],
                                         binrow_sb[:, ts(cc, 128)], s16,
                                         start=False, stop=True)
                    # muS = mu_preT / 64 (fp16); n2 = sum_c muS^2 (PE)
                    muS = wpool.tile([128, CC, K], F16, tag=f"muS{s}",
                                     name=f"muS{s}")
                    nc.scalar.activation(muS, mu_ps, AF.Copy, bias=0.0,
                                         scale=1.0 / 64.0)
                    sq = wpool.tile([128, CC, K], F16, tag=f"sq{s}",
                                    name=f"sq{s}")
                    nc.vector.tensor_tensor(out=sq, in0=muS, in1=muS,
                                            op=ALU.mult)
                    n2_ps = rowpool.tile([1, K], F32, tag="row",
                                         name=f"n2_ps{s}")
                    for cc in range(CC):
                        nc.tensor.matmul(n2_ps, ones_col, sq[:, cc, :],
                                         start=(cc == 0), stop=(cc == CC - 1))
                    n2f = wpool.tile([1, K], F32, tag=f"n2f{s}", name=f"n2f{s}")
                    nc.vector.tensor_copy(n2f, n2_ps)
                    # Quake rsqrt on the thin row (no ACT tables)
                    yy = wpool.tile([1, K], F32, tag=f"yy{s}", name=f"yy{s}")
                    ti = wpool.tile([1, K], mybir.dt.int32, tag=f"ti{s}",
                                    name=f"ti{s}")
                    nc.vector.tensor_scalar(ti, n2f.bitcast(mybir.dt.int32), 1,
                                            None, op0=ALU.logical_shift_right)
                    nc.vector.tensor_scalar(ti, ti, -1, None,
                                            op0=ALU.bitwise_xor)
                    nc.vector.tensor_scalar(yy.bitcast(mybir.dt.int32), ti,
                                            0x5f3759df + 1, None, op0=ALU.add)
                    tb = wpool.tile([1, K], F32, tag=f"tb{s}", name=f"tb{s}")
                    for _ in range(2):
                        nc.vector.tensor_tensor(out=tb, in0=yy, in1=yy,
                                                op=ALU.mult)
                        nc.vector.tensor_tensor(out=tb, in0=tb, in1=n2f,
                                                op=ALU.mult)
                        nc.vector.tensor_scalar(tb, tb, -0.5, 1.5,
                                                op0=ALU.mult, op1=ALU.add)
                        nc.vector.tensor_tensor(out=yy, in0=yy, in1=tb,
                                                op=ALU.mult)
                    rn16 = wpool.tile([1, K], F16, tag=f"rn{s}", name=f"rn{s}")
                    nc.vector.tensor_copy(rn16, yy)
                    # broadcast rn to 128 partitions via PE outer product
                    rnb_ps = rowpool.tile([128, K], F32, tag="row",
                                          name=f"rnb_ps{s}")
                    nc.tensor.matmul(rnb_ps, ones_row, rn16, start=True,
                                     stop=True)
                    rnb16 = wpool.tile([128, K], F16, tag=f"rnb{s}",
                                       name=f"rnb{s}")
                    nc.scalar.copy(rnb16, rnb_ps)
                    muT_new = wpool.tile([128, CC, K], F16, tag=f"muT{s}",
                                         name=f"muT{s}")
                    nc.vector.tensor_tensor(
                        out=muT_new, in0=muS,
                        in1=bcast(rnb16, [[0, CC], [1, K]]),
                        op=ALU.mult)
                    muT[s] = muT_new
                    if it < T - 1:
                        m2t_ps = scpool.tile([128, CC, K], F32, tag=f"sc{s}",
                                             name=f"m2t_ps{s}")
                        for cc in range(CC):
                            for oc in range(CC):
                                nc.tensor.matmul(
                                    m2t_ps[:, cc, :],
                                    w_sb[:, oc, ts(cc, 128)],
                                    muT[s][:, oc, :],
                                    start=(oc == 0), stop=(oc == CC - 1))
                        m2t_sb = wpool.tile([128, CC, K], F16, tag=f"m2t{s}",
                                            name=f"m2t_sb{s}")
                        nc.scalar.copy(m2t_sb, m2t_ps)
                        m2t[s] = m2t_sb
                        beta_ps = rowpool.tile([1, K], F32, tag="row",
                                               name=f"beta_ps{s}")
                        for oc in range(CC):
                            nc.tensor.matmul(beta_ps, bin_sb[:, oc, :],
                                             muT[s][:, oc, :],
                                             start=(oc == 0),
                                             stop=(oc == CC - 1))
                        b16 = wpool.tile([1, K], F16, tag=f"b16_{s}",
                                         name=f"b16_{s}")
                        nc.vector.tensor_copy(b16, beta_ps)
                        beta16[s] = b16

            # ---- output path: out = Z @ (mu wot); host adds x + S ----
            for s in range(SPC):
                m3_ps = scpool.tile([K, C], F32, tag=f"sc{s}", name=f"m3_ps{s}")
                for cc in range(CC):
                    nc.tensor.matmul(m3_ps, muT[s][:, cc, :], wot_sb[:, cc, :],
                                     start=(cc == 0), stop=(cc == CC - 1))
                m3s = wpool.tile([K, C], F16, tag=f"m3s{s}", bufs=1,
                                 name=f"m3s{s}")
                nc.vector.tensor_copy(m3s, m3_ps)
                ZT = wpool.tile([K, N], F16, tag=f"ZT{s}", bufs=1,
                                name=f"ZT{s}")
                for g in range(NT // 4):
                    zt_ps = scpool.tile([K, 4, 128], F16, tag=f"sc{s}",
                                        name=f"zt_ps{s}_{g}")
                    for j in range(4):
                        nc.tensor.transpose(zt_ps[:, j, :], Z[s][:, g * 4 + j, :],
                                            ident)
                    dst = ZT[:, ts(g, 512)].rearrange("p (a b) -> p a b", a=4)
                    if g % 2 == 0:
                        nc.vector.tensor_copy(dst, zt_ps)
                    else:
                        nc.scalar.copy(dst, zt_ps)
                for oc in range(CC):
                    for half in range(2):
                        osb = opool.tile([128, 2048], F16, tag="osb",
                                         name=f"osb{s}_{oc}_{half}")
                        for nj in range(4):
                            nk = half * 4 + nj
                            o2 = lgpool.tile([128, 512], F32,
                                             tag=f"lg{(s + nk) % SPC}",
                                             name=f"o2_{s}_{oc}_{nk}")
                            nc.tensor.matmul(o2, m3s[:, ts(oc, 128)],
                                             ZT[:, ts(nk, 512)],
                                             start=True, stop=True)
                            dst = osb[:, ts(nj, 512)]
                            if (oc * 8 + nk) % 2 == 0:
                                nc.vector.tensor_copy(dst, o2)
                            else:
                                nc.scalar.copy(dst, o2)
                        nc.sync.dma_start(out=outp[s, oc, :, ts(half, 2048)],
                                          in_=osb)

    nc.compile()
    return nc


_NC_CACHE = None
_RUN_KWARGS: dict = {}   # extra kwargs for run_bass_kernel_spmd (e.g. trace=True)
_LAST_RESULTS = None     # BassKernelResults of the most recent run


def _get_nc():
    global _NC_CACHE
    if _NC_CACHE is None:
        _NC_CACHE = build_bass()
    return _NC_CACHE


def kernel(x, w_in, b_in, w_out, b_out, gamma, beta, running_mean, running_var,
           bases):
    x = np.asarray(x, np.float32)
    w_in = np.asarray(w_in, np.float32)
    b_in = np.asarray(b_in, np.float32)
    w_out = np.asarray(w_out, np.float32)
    b_out = np.asarray(b_out, np.float32)
    gamma = np.asarray(gamma, np.float32)
    beta = np.asarray(beta, np.float32)
    running_mean = np.asarray(running_mean, np.float32)
    running_var = np.asarray(running_var, np.float32)
    bases = np.asarray(bases, np.float32)

    inv = gamma / np.sqrt(running_var + BN_EPS)
    S = b_out * inv + beta - running_mean * inv
    wot = (w_out * inv[:, None]).T                      # (C, O)
    m2t0 = w_in.T @ bases.T                             # (C, K)
    beta0 = (b_in @ bases.T).reshape(1, K)              # (1, K)

    xr = x.reshape(B, C, N)
    x16 = np.ascontiguousarray(
        xr.reshape(B, CC, 128, N).transpose(0, 2, 1, 3)).astype(np.float16)
    xt16 = np.ascontiguousarray(
        xr.transpose(0, 2, 1).reshape(B, NT, 128, C).transpose(0, 2, 1, 3)
    ).astype(np.float16)

    chunk = lambda a, f: a.reshape(CC, 128, f).transpose(1, 0, 2)
    wcat = np.ascontiguousarray(np.concatenate([
        chunk(w_in, C), chunk(np.ascontiguousarray(w_in.T), C),
        chunk(np.ascontiguousarray(wot), C), chunk(b_in, 1),
        chunk(np.ascontiguousarray(m2t0), K),
    ], axis=2)).astype(np.float16)
    binrow16 = b_in.reshape(1, C).astype(np.float16)
    beta0r = beta0.astype(np.float16)

    in_maps = []
    for core in range(NCORES):
        sl = slice(core * SPC, (core + 1) * SPC)
        in_maps.append({
            "x16": x16[sl], "xt16": xt16[sl],
            "wcat": wcat, "binrow": binrow16, "beta0r": beta0r,
        })

    nc = _get_nc()
    res = bass_utils.run_bass_kernel_spmd(nc, in_maps, core_ids=list(range(NCORES)),
                                          **_RUN_KWARGS)
    global _LAST_RESULTS
    _LAST_RESULTS = res
    out = np.empty((B, C, N), np.float32)
    for core in range(NCORES):
        o = res.results[core]["outp"]                   # (SPC, CC, 128, N) f16
        out[core * SPC:(core + 1) * SPC] = o.astype(np.float32).reshape(SPC, C, N)
    out += xr + S[None, :, None]                        # residual + BN shift
    return out.reshape(B, C, H, W)


# revision 19
# speedup vs baseline: 1.2009x; 1.2009x over previous
"""TRN2 Bass kernel for nn_EMAModule (EM attention module).

Computation (per sample):
    xf = conv1x1(x, w_in, b_in); T=3 EM iterations (softmax E-step over K=64
    bases, L2-normalized M-step); reconstruct; conv1x1(w_out, b_out);
    eval-BatchNorm; +residual.

Restructuring (validated vs reference to ~1e-4 rel):
    - xf never materialized: logits come from x via folded m2t = w_in^T mu
      (C,K) plus a bias row beta_k = b_in.mu appended as a 1-row matmul into
      the same PSUM accumulation (no eb broadcast multiply needed).
    - M-step normalize-without-divide: mu = normalize(G w_in^T + s (x) b_in),
      since the /(s+eps) scale cancels under L2 normalization. s enters as a
      rank-1 single-row matmul. G is computed directly transposed
      (GT_ck = sum_n XT[n,c] Z[n,k], F=64 matmuls) so no PE transposes or
      extra copies; the norm is a PE ones-matmul over muS^2 with a Quake
      rsqrt (bit-trick + 2 Newton steps) on a thin row.
    - Output path: recon matmul only. BN shift S, b_out and the residual are
      added on the host (out = dev_fp16 + x + S), so the device PSUM->SBUF
      move is a plain fp16 cast copy and output DMA bytes are halved.
    - All matmul operands fp16 (PE 1 cycle per output column); statistics
      accumulate in fp32 PSUM.

Sharding: data-parallel over batch, 2 samples per NeuronCore on 8 cores.
"""
import numpy as np

import concourse.bacc as bacc
import concourse.bass as bass
import concourse.tile as tile
from concourse import mybir
from concourse import bass_utils
from concourse.masks import make_identity

F32 = mybir.dt.float32
F16 = mybir.dt.float16
AF = mybir.ActivationFunctionType
ALU = mybir.AluOpType

B, C, H, W, K = 16, 512, 64, 64, 64
N = H * W                 # 4096
NCORES = 8
SPC = B // NCORES         # samples per core = 2
T = 3
BN_EPS = 1e-5
EXP_SHIFT = -7.5          # exp(logit + shift): cancels in softmax ratio,
                          # keeps fp16 row sums < 3e4 (logits <= 13.2)
CC = C // 128             # 4 channel chunks
NT = N // 128             # 32 n-tiles
NQ = 4                    # logits quarters
NTQ = NT // NQ            # 8 n-tiles per quarter
NK = N // 512             # 8 n-chunks of 512
WCATW = 3 * C + 1 + K     # w | wt | wot | bin col | m2t0


def ts(i, sz):
    return bass.ts(i, sz)


def bcast(ap, axes):
    """AP with given (stride, num) list appended after the partition dim."""
    return bass.AP(tensor=ap.tensor, offset=ap.offset, ap=[ap.ap[0]] + axes)


def build_bass():
    nc = bacc.Bacc("TRN2", target_bir_lowering=False, debug=False,
                   num_devices=NCORES)
    dram = lambda name, shape, dt, kind: nc.dram_tensor(name, shape, dt, kind=kind).ap()
    x16 = dram("x16", [SPC, 128, CC, N], F16, "ExternalInput")
    xt16 = dram("xt16", [SPC, 128, NT, C], F16, "ExternalInput")
    wcat = dram("wcat", [128, CC, WCATW], F16, "ExternalInput")
    binrow = dram("binrow", [1, C], F16, "ExternalInput")    # b_in row
    beta0r = dram("beta0r", [1, K], F16, "ExternalInput")    # b_in . bases
    outp = dram("outp", [SPC, CC, 128, N], F16, "ExternalOutput")

    with tile.TileContext(nc) as tc:
        with (
            tc.tile_pool(name="const", bufs=1) as cpool,
            tc.tile_pool(name="xin", bufs=2) as xpool,
            tc.tile_pool(name="xt", bufs=2) as xtpool,
            tc.tile_pool(name="work", bufs=2) as wpool,
            tc.tile_pool(name="outsb", bufs=2) as opool,
            tc.tile_pool(name="lg", bufs=2, space="PSUM") as lgpool,
            tc.tile_pool(name="sc", bufs=1, space="PSUM") as scpool,
            tc.tile_pool(name="srow", bufs=2, space="PSUM") as rowpool,
        ):
            # ---- constants ----
            wcat_sb = cpool.tile([128, CC, WCATW], F16)
            nc.sync.dma_start(out=wcat_sb, in_=wcat)
            w_sb = wcat_sb[:, :, 0:C]
            wt_sb = wcat_sb[:, :, C:2 * C]
            wot_sb = wcat_sb[:, :, 2 * C:3 * C]
            bin_sb = wcat_sb[:, :, 3 * C:3 * C + 1]
            m2t0_sb = wcat_sb[:, :, 3 * C + 1:3 * C + 1 + K]
            binrow_sb = cpool.tile([1, C], F16)
            nc.sync.dma_start(out=binrow_sb, in_=binrow)
            beta0_sb = cpool.tile([1, K], F16)
            nc.sync.dma_start(out=beta0_sb, in_=beta0r)
            ident = cpool.tile([128, 128], F16)
            make_identity(nc, ident)
            ones_row = cpool.tile([1, 128], F16)
            nc.vector.memset(ones_row, 1.0)
            ones_col = cpool.tile([128, 1], F16)
            nc.vector.memset(ones_col, 1.0)
            expbias = cpool.tile([128, 1], F32)
            nc.vector.memset(expbias, EXP_SHIFT)

            # per-sample input loads (X then XT per sample so sample 0's
            # M-step is fed before sample 1 finishes streaming)
            X, XT = [None] * SPC, [None] * SPC
            for s in range(SPC):
                X[s] = xpool.tile([128, CC, N], F16, tag="x", name=f"X{s}")
                for q in range(NQ):
                    nc.sync.dma_start(out=X[s][:, :, ts(q, N // NQ)],
                                      in_=x16[s][:, :, ts(q, N // NQ)])
                XT[s] = xtpool.tile([128, NT, C], F16, tag="xt", name=f"XT{s}")
                for q in range(NQ):
                    nc.sync.dma_start(out=XT[s][:, ts(q, NTQ), :],
                                      in_=xt16[s][:, ts(q, NTQ), :])

            m2t = [m2t0_sb] * SPC         # (128, CC, K) fp16
            beta16 = [beta0_sb] * SPC     # (1, K) fp16
            Z = [None] * SPC
            muT = [None] * SPC

            def phase_a(it, s):
                # ---- phase A: logits (+beta row), exp, row sums, Z ----
                if True:
                    E = wpool.tile([128, NT, K], F16, tag=f"E{s}", bufs=1,
                                   name=f"E{s}")
                    r = wpool.tile([128, NT], F16, tag=f"r{s}", name=f"r{s}")
                    rv = wpool.tile([128, NT], F16, tag=f"rv{s}", name=f"rv{s}")
                    Z[s] = wpool.tile([128, NT, K], F16, tag=f"Z{s}", bufs=1,
                                      name=f"Z_{s}")
                    for q in range(NQ):
                        lg = lgpool.tile([128, NTQ, K], F32, tag=f"lg{s}",
                                         name=f"lg{s}_{q}")
                        for t8 in range(NTQ):
                            t = q * NTQ + t8
                            for cc in range(CC):
                                nc.tensor.matmul(
                                    lg[:, t8, :],
                                    X[s][:, cc, ts(t, 128)],
                                    m2t[s][:, cc, :],
                                    start=(cc == 0), stop=False)
                            nc.tensor.matmul(lg[:, t8, :], ones_row,
                                             beta16[s], start=False, stop=True)
                        Eq = E[:, ts(q, NTQ), :]
                        nc.scalar.activation(Eq, lg, AF.Exp,
                                             bias=expbias, scale=1.0)
                        rq = r[:, ts(q, NTQ)]
                        with nc.allow_low_precision("fp16 softmax denom"):
                            nc.vector.reduce_sum(rq, Eq,
                                                 axis=mybir.AxisListType.X)
                        rvq = rv[:, ts(q, NTQ)]
                        with nc.allow_low_precision("fp16 softmax recip"):
                            nc.vector.reciprocal(rvq, rq)
                        nc.vector.tensor_tensor(
                            out=Z[s][:, ts(q, NTQ), :], in0=Eq,
                            in1=bcast(rvq, [[1, NTQ], [0, K]]),
                            op=ALU.mult)

            def phase_b(it, s):
                # ---- phase B: M-step ----
                # GT_ck = sum_n XT[n,c] Z[n,k]  (F=64 matmuls)
                if True:
                    GT_ps = scpool.tile([128, CC, K], F32, tag=f"sc{s}",
                                        name=f"GT_ps{s}")
                    s_ps = rowpool.tile([1, K], F32, tag="row", name=f"s_ps{s}")
                    for cc in range(CC):
                        for t in range(NT):
                            nc.tensor.matmul(GT_ps[:, cc, :],
                                             XT[s][:, t, ts(cc, 128)],
                                             Z[s][:, t, :],
                                             start=(t == 0), stop=(t == NT - 1))
                    for t in range(NT):
                        nc.tensor.matmul(s_ps, ones_col, Z[s][:, t, :],
                                         start=(t == 0), stop=(t == NT - 1))
                    GT_sb = wpool.tile([128, CC, K], F16, tag=f"GT{s}",
                                       name=f"GT_sb{s}")
                    nc.scalar.copy(GT_sb, GT_ps)
                    s16 = wpool.tile([1, K], F16, tag=f"s16_{s}", name=f"s16_{s}")
                    nc.vector.tensor_copy(s16, s_ps)
                    # mu_preT = w_in GT + b_in (x) s   (c-part, K)
                    mu_ps = scpool.tile([128, CC, K], F32, tag=f"sc{s}",
                                        name=f"mu_ps{s}")
                    for cc in range(CC):
                        for cp in range(CC):
                            nc.tensor.matmul(mu_ps[:, cc, :],
                                             wt_sb[:, cp, ts(cc, 128)],
                                             GT_sb[:, cp, :],
                                             start=(cp == 0), stop=False)
                        nc.tensor.matmul(mu_ps[:, cc, :],
                                         binrow_sb[:, ts(cc, 128)], s16,
                                         start=False, stop=True)
                    # muS = mu_preT / 64 (fp16); n2 = sum_c muS^2 (PE)
                    muS = wpool.tile([128, CC, K], F16, tag=f"muS{s}",
                                     name=f"muS{s}")
                    nc.scalar.activation(muS, mu_ps, AF.Copy, bias=0.0,
                                         scale=1.0 / 64.0)
                    sq = wpool.tile([128, CC, K], F16, tag=f"sq{s}",
                                    name=f"sq{s}")
                    nc.vector.tensor_tensor(out=sq, in0=muS, in1=muS,
                                            op=ALU.mult)
                    n2_ps = rowpool.tile([1, K], F32, tag="row",
                                         name=f"n2_ps{s}")
                    for cc in range(CC):
                        nc.tensor.matmul(n2_ps, ones_col, sq[:, cc, :],
                                         start=(cc == 0), stop=(cc == CC - 1))
                    n2f = wpool.tile([1, K], F32, tag=f"n2f{s}", name=f"n2f{s}")
                    nc.vector.tensor_copy(n2f, n2_ps)
                    # Quake rsqrt on the thin row (no ACT tables)
                    yy = wpool.tile([1, K], F32, tag=f"yy{s}", name=f"yy{s}")
                    ti = wpool.tile([1, K], mybir.dt.int32, tag=f"ti{s}",
                                    name=f"ti{s}")
                    nc.vector.tensor_scalar(ti, n2f.bitcast(mybir.dt.int32), 1,
                                            None, op0=ALU.logical_shift_right)
                    nc.vector.tensor_scalar(ti, ti, -1, None,
                                            op0=ALU.bitwise_xor)
                    nc.vector.tensor_scalar(yy.bitcast(mybir.dt.int32), ti,
                                            0x5f3759df + 1, None, op0=ALU.add)
                    tb = wpool.tile([1, K], F32, tag=f"tb{s}", name=f"tb{s}")
                    for _ in range(2):
                        nc.vector.tensor_tensor(out=tb, in0=yy, in1=yy,
                                                op=ALU.mult)
                        nc.vector.tensor_tensor(out=tb, in0=tb, in1=n2f,
                                                op=ALU.mult)
                        nc.vector.tensor_scalar(tb, tb, -0.5, 1.5,
                                                op0=ALU.mult, op1=ALU.add)
                        nc.vector.tensor_tensor(out=yy, in0=yy, in1=tb,
                                                op=ALU.mult)
                    rn16 = wpool.tile([1, K], F16, tag=f"rn{s}", name=f"rn{s}")
                    nc.vector.tensor_copy(rn16, yy)
                    # broadcast rn to 128 partitions via PE outer product
                    rnb_ps = rowpool.tile([128, K], F32, tag="row",
                                          name=f"rnb_ps{s}")
                    nc.tensor.matmul(rnb_ps, ones_row, rn16, start=True,
                                     stop=True)
                    rnb16 = wpool.tile([128, K], F16, tag=f"rnb{s}",
                                       name=f"rnb{s}")
                    nc.scalar.copy(rnb16, rnb_ps)
                    muT_new = wpool.tile([128, CC, K], F16, tag=f"muT{s}",
                                         name=f"muT{s}")
                    nc.vector.tensor_tensor(
                        out=muT_new, in0=muS,
                        in1=bcast(rnb16, [[0, CC], [1, K]]),
                        op=ALU.mult)
                    muT[s] = muT_new
                    if it < T - 1:
                        m2t_ps = scpool.tile([128, CC, K], F32, tag=f"sc{s}",
                                             name=f"m2t_ps{s}")
                        beta_ps = rowpool.tile([1, K], F32, tag="row",
                                               name=f"beta_ps{s}")
                        for cc in range(CC):
                            for oc in range(CC):
                                nc.tensor.matmul(
                                    m2t_ps[:, cc, :],
                                    w_sb[:, oc, ts(cc, 128)],
                                    muT[s][:, oc, :],
                                    start=(oc == 0), stop=(oc == CC - 1))
                        for oc in range(CC):
                            nc.tensor.matmul(beta_ps, bin_sb[:, oc, :],
                                             muT[s][:, oc, :],
                                             start=(oc == 0),
                                             stop=(oc == CC - 1))
                        m2t_sb = wpool.tile([128, CC, K], F16, tag=f"m2t{s}",
                                            name=f"m2t_sb{s}")
                        nc.scalar.copy(m2t_sb, m2t_ps)
                        m2t[s] = m2t_sb
                        b16 = wpool.tile([1, K], F16, tag=f"b16_{s}",
                                         name=f"b16_{s}")
                        nc.vector.tensor_copy(b16, beta_ps)
                        beta16[s] = b16

            # ---- output path: out = Z @ (mu wot); host adds x + S ----
            def out_phase(s):
                m3_ps = scpool.tile([K, C], F32, tag=f"sc{s}", name=f"m3_ps{s}")
                for cc in range(CC):
                    nc.tensor.matmul(m3_ps, muT[s][:, cc, :], wot_sb[:, cc, :],
                                     start=(cc == 0), stop=(cc == CC - 1))
                m3s = wpool.tile([K, C], F16, tag=f"m3s{s}", bufs=1,
                                 name=f"m3s{s}")
                nc.vector.tensor_copy(m3s, m3_ps)
                ZT = wpool.tile([K, N], F16, tag=f"ZT{s}", bufs=1,
                                name=f"ZT{s}")
                for g in range(NT // 4):
                    # alternate PSUM tags so transposes overlap the copies
                    zt_ps = scpool.tile([K, 4, 128], F16,
                                        tag=f"sc{s if g % 2 == 0 else 1 - s}",
                                        name=f"zt_ps{s}_{g}")
                    for j in range(4):
                        nc.tensor.transpose(zt_ps[:, j, :], Z[s][:, g * 4 + j, :],
                                            ident)
                    dst = ZT[:, ts(g, 512)].rearrange("p (a b) -> p a b", a=4)
                    if g % 2 == 0:
                        nc.vector.tensor_copy(dst, zt_ps)
                    else:
                        nc.scalar.copy(dst, zt_ps)
                for oc in range(CC):
                    for half in range(2):
                        osb = opool.tile([128, 2048], F16, tag="osb",
                                         name=f"osb{s}_{oc}_{half}")
                        for nj in range(4):
                            nk = half * 4 + nj
                            o2 = lgpool.tile([128, 512], F32,
                                             tag=f"lg{(s + nk) % SPC}",
                                             name=f"o2_{s}_{oc}_{nk}")
                            nc.tensor.matmul(o2, m3s[:, ts(oc, 128)],
                                             ZT[:, ts(nk, 512)],
                                             start=True, stop=True)
                            dst = osb[:, ts(nj, 512)]
                            if (oc * 8 + nk) % 2 == 0:
                                nc.vector.tensor_copy(dst, o2)
                            else:
                                nc.scalar.copy(dst, o2)
                        nc.sync.dma_start(out=outp[s, oc, :, ts(half, 2048)],
                                          in_=osb)

            # drive: on the last iteration interleave sample 0's output with
            # sample 1's M-step so output copies overlap M-step PE work
            for it in range(T):
                for s in range(SPC):
                    phase_a(it, s)
                if it < T - 1:
                    for s in range(SPC):
                        phase_b(it, s)
                else:
                    for s in range(SPC):
                        phase_b(it, s)
                        out_phase(s)

    nc.compile()
    return nc


_NC_CACHE = None
_RUN_KWARGS: dict = {}   # extra kwargs for run_bass_kernel_spmd (e.g. trace=True)
_LAST_RESULTS = None     # BassKernelResults of the most recent run


def _get_nc():
    global _NC_CACHE
    if _NC_CACHE is None:
        _NC_CACHE = build_bass()
    return _NC_CACHE


def kernel(x, w_in, b_in, w_out, b_out, gamma, beta, running_mean, running_var,
           bases):
    x = np.asarray(x, np.float32)
    w_in = np.asarray(w_in, np.float32)
    b_in = np.asarray(b_in, np.float32)
    w_out = np.asarray(w_out, np.float32)
    b_out = np.asarray(b_out, np.float32)
    gamma = np.asarray(gamma, np.float32)
    beta = np.asarray(beta, np.float32)
    running_mean = np.asarray(running_mean, np.float32)
    running_var = np.asarray(running_var, np.float32)
    bases = np.asarray(bases, np.float32)

    inv = gamma / np.sqrt(running_var + BN_EPS)
    S = b_out * inv + beta - running_mean * inv
    wot = (w_out * inv[:, None]).T                      # (C, O)
    m2t0 = w_in.T @ bases.T                             # (C, K)
    beta0 = (b_in @ bases.T).reshape(1, K)              # (1, K)

    xr = x.reshape(B, C, N)
    x16 = np.ascontiguousarray(
        xr.reshape(B, CC, 128, N).transpose(0, 2, 1, 3)).astype(np.float16)
    xt16 = np.ascontiguousarray(
        xr.transpose(0, 2, 1).reshape(B, NT, 128, C).transpose(0, 2, 1, 3)
    ).astype(np.float16)

    chunk = lambda a, f: a.reshape(CC, 128, f).transpose(1, 0, 2)
    wcat = np.ascontiguousarray(np.concatenate([
        chunk(w_in, C), chunk(np.ascontiguousarray(w_in.T), C),
        chunk(np.ascontiguousarray(wot), C), chunk(b_in, 1),
        chunk(np.ascontiguousarray(m2t0), K),
    ], axis=2)).astype(np.float16)
    binrow16 = b_in.reshape(1, C).astype(np.float16)
    beta0r = beta0.astype(np.float16)

    in_maps = []
    for core in range(NCORES):
        sl = slice(core * SPC, (core + 1) * SPC)
        in_maps.append({
            "x16": x16[sl], "xt16": xt16[sl],
            "wcat": wcat, "binrow": binrow16, "beta0r": beta0r,
        })

    nc = _get_nc()
    res = bass_utils.run_bass_kernel_spmd(nc, in_maps, core_ids=list(range(NCORES)),
                                          **_RUN_KWARGS)
    global _LAST_RESULTS
    _LAST_RESULTS = res
    out = np.empty((B, C, N), np.float32)
    for core in range(NCORES):
        o = res.results[core]["outp"]                   # (SPC, CC, 128, N) f16
        out[core * SPC:(core + 1) * SPC] = o.astype(np.float32).reshape(SPC, C, N)
    out += xr + S[None, :, None]                        # residual + BN shift
    return out.reshape(B, C, H, W)
